# revision 1
# baseline (speedup 1.0000x reference)
"""CrissCrossAttention TRN2 kernel.

Shapes (hardcoded): x [16, 512, 96, 96] fp32, Wq/Wk [64, 512], Wv [512, 512],
biases, gamma [1]. 8 NeuronCores, data-parallel over batch (2 batches/core).

Math per batch (reference):
  q = Wq@x, k = Wk@x, val = Wv@x (1x1 convs over channel dim)
  eH[v,t,u] = sum_c q[c,v,t] k[c,u,t]  (diag u==v masked out)
  eW[v,t,s] = sum_c q[c,v,t] k[c,v,s]
  att = softmax(concat(eH, eW))  ->  out = gamma*(outH+outW) + x

Device strategy (per core, per batch):
  - x uploaded as fp16 [512, 9216]; fused q|k projection with stationary
    [WqT|WkT] (M=128), valT = x^T@WvT in [vt, c] layout -> DRAM scratch (bf16).
  - Scores computed TRANSPOSED (key index on partitions): PHt[u,v] =
    exp(k_t^T q_t), PWv[s,t] = exp(k_v^T q_v); these are directly lhsT-ready
    for the output matmuls (no per-tile transposes). Softmax max-subtraction
    is skipped: scores are bounded (~50) so exp stays in fp32/bf16 range.
    Score tiles are processed 4-per-PSUM-bank so one exp / one mask-multiply
    covers 4 tiles.
  - Denominators via ones-matmuls (column sums across partitions land in
    PSUM columns); R = gamma / (denH + denW) with one 96x96 PE transpose.
  - Output matmuls produce [spatial, channel] tiles with N=512 (full rate);
    normalization fused into the PSUM eviction as per-partition scale.
  - outH partials (tiled by t) and outW partials (tiled by v) cannot be
    summed on device without a transpose, so both are written as fp16
    streams; host does (OH + OW).T + x.
  - DMAs are batched (8 tiles per load/store, 4 per valT store) and spread:
    loads on SP (HWDGE), stores on GPSIMD (SWDGE) to cut per-DMA issue
    overhead off the critical path.
"""

import numpy as np
import ml_dtypes

import concourse.bacc as bacc
import concourse.bass as bass
import concourse.tile as tile
from concourse import mybir
from concourse.bass_utils import run_bass_kernel_spmd

F32 = mybir.dt.float32
F16 = mybir.dt.float16
BF16 = mybir.dt.bfloat16
EXP = mybir.ActivationFunctionType.Exp
COPY = mybir.ActivationFunctionType.Copy

B, C, V, T = 16, 512, 96, 96
C8 = 64
S = V * T            # 9216 spatial positions
NCORES = 8
BPC = B // NCORES    # batches per core
NCI = C // 128       # cin chunks
NJ = S // 512        # 512-wide column blocks
NVT = 512 // 128     # 128-row valT chunks per block
SG = 4               # score tiles grouped per PSUM bank
OG = 8               # out/load tiles grouped per DMA

_CACHE = {}


def _build(has_bv: bool, has_bqk: bool, repeat: int = 1):
    nc = bacc.Bacc("TRN2", target_bir_lowering=False, debug=False)

    xh = nc.dram_tensor("xh", [BPC, C, S], F16, kind="ExternalInput").ap()
    wqk = nc.dram_tensor("wqk", [NCI, 128, 128], F16, kind="ExternalInput").ap()
    wv4 = nc.dram_tensor("wv4", [NCI, 128, C], F16, kind="ExternalInput").ap()
    bqk = nc.dram_tensor("bqk", [128, 1], F32, kind="ExternalInput").ap()
    bv2 = nc.dram_tensor("bv2", [1, C], F32, kind="ExternalInput").ap()
    gam = nc.dram_tensor("gam", [1, 1], F32, kind="ExternalInput").ap()
    msk = nc.dram_tensor("msk", [V, SG * V], BF16, kind="ExternalInput").ap()
    i96 = nc.dram_tensor("i96", [V, V], F32, kind="ExternalInput").ap()
    oh_d = nc.dram_tensor("oh", [BPC, S, C], F16, kind="ExternalOutput").ap()
    ow_d = nc.dram_tensor("ow", [BPC, S, C], F16, kind="ExternalOutput").ap()
    vtd = nc.dram_tensor("vtd", [BPC, S, C], BF16).ap()

    with tile.TileContext(nc) as tc:
        with (
            tc.tile_pool(name="const", bufs=1) as cst,
            tc.tile_pool(name="xts", bufs=8) as xts,
            tc.tile_pool(name="qkp", bufs=1) as qkp,
            tc.tile_pool(name="php", bufs=1) as php,
            tc.tile_pool(name="stg", bufs=3) as stg,
            tc.tile_pool(name="den", bufs=2) as den,
            tc.tile_pool(name="ps", bufs=1, space="PSUM") as psp,
        ):
            wqk_sb = cst.tile([128, NCI, 128], F16)
            nc.sync.dma_start(out=wqk_sb, in_=wqk.rearrange("a p b -> p a b"))
            wv_sb = cst.tile([128, NCI, C], F16)
            nc.sync.dma_start(out=wv_sb, in_=wv4.rearrange("a p b -> p a b"))
            bqk_sb = cst.tile([128, 1], F32)
            nc.sync.dma_start(out=bqk_sb, in_=bqk)
            gam_sb = cst.tile([V, 1], F32)
            nc.sync.dma_start(out=gam_sb, in_=gam.to_broadcast([V, 1]))
            msk_sb = cst.tile([V, SG, V], BF16)
            nc.sync.dma_start(out=msk_sb,
                              in_=msk.rearrange("p (a b) -> p a b", a=SG))
            i96_sb = cst.tile([V, V], F32)
            nc.sync.dma_start(out=i96_sb, in_=i96)
            ones_sb = cst.tile([V, 1], BF16)
            nc.vector.memset(ones_sb, 1.0)
            if has_bv:
                bv_sb = cst.tile([128, C], F32)
                nc.sync.dma_start(out=bv_sb, in_=bv2.to_broadcast([128, C]))

            for b in [bb for _ in range(repeat) for bb in range(BPC)]:
                x_b = xh[b].rearrange("(ci p) s -> p ci s", p=128)

                # ---- phase A: q|k projection + valT ----
                qk = qkp.tile([C8, 2, S], F16, tag="qk")
                for j in range(NJ):
                    xt = xts.tile([128, NCI, 512], F16, tag="xt")
                    nc.sync.dma_start(
                        out=xt, in_=x_b[:, :, j * 512 : (j + 1) * 512]
                    )
                    pq = psp.tile([128, 512], F32, tag="big", bufs=3)
                    for ci in range(NCI):
                        nc.tensor.matmul(
                            pq, wqk_sb[:, ci, :], xt[:, ci, :],
                            start=(ci == 0), stop=(ci == NCI - 1),
                        )
                    js = slice(j * 512, (j + 1) * 512)
                    if has_bqk:
                        iden = mybir.ActivationFunctionType.Identity
                        nc.scalar.activation(
                            out=qk[:, 0, js], in_=pq[0:C8, :], func=iden,
                            bias=bqk_sb[0:C8],
                        )
                        nc.scalar.activation(
                            out=qk[:, 1, js], in_=pq[C8:128, :], func=iden,
                            bias=bqk_sb[C8:128],
                        )
                    else:
                        nc.scalar.activation(
                            out=qk[:, 0, js], in_=pq[0:C8, :], func=COPY
                        )
                        nc.scalar.activation(
                            out=qk[:, 1, js], in_=pq[C8:128, :], func=COPY
                        )
                    vs = stg.tile([128, NVT, C], BF16, tag="vts", bufs=3)
                    for jj in range(NVT):
                        pv = psp.tile([128, 512], F32, tag="big", bufs=3)
                        for ci in range(NCI):
                            nc.tensor.matmul(
                                pv,
                                xt[:, ci, jj * 128 : (jj + 1) * 128],
                                wv_sb[:, ci, :],
                                start=(ci == 0), stop=(ci == NCI - 1),
                            )
                        if has_bv:
                            nc.vector.tensor_add(vs[:, jj, :], pv, bv_sb)
                        elif jj % 2 == 0:
                            nc.vector.tensor_copy(vs[:, jj, :], pv)
                        else:
                            nc.scalar.activation(
                                out=vs[:, jj, :], in_=pv, func=COPY
                            )
                    nc.gpsimd.dma_start(
                        out=vtd[b, j * 512 : (j + 1) * 512, :].rearrange(
                            "(cb p) c -> p cb c", p=128
                        ),
                        in_=vs,
                    )

                q3 = qk[:, 0, :].rearrange("p (v t) -> p v t", t=T)
                k3 = qk[:, 1, :].rearrange("p (v t) -> p v t", t=T)

                # ---- phase B: transposed scores + exp (grouped by SG) ----
                ph = php.tile([V, T, V], BF16, tag="ph")
                pw = php.tile([V, V, T], BF16, tag="pw")
                for t0 in range(0, T, SG):
                    ps = psp.tile([V, SG, V], F32, tag="sc", bufs=3)
                    for g in range(SG):
                        nc.tensor.matmul(
                            ps[:, g, :], k3[:, :, t0 + g], q3[:, :, t0 + g],
                            start=True, stop=True,
                        )
                    nc.scalar.activation(
                        out=ph[:, t0 : t0 + SG, :],
                        in_=ps.rearrange("p a b -> p (a b)"),
                        func=EXP,
                    )
                    nc.vector.tensor_mul(
                        ph[:, t0 : t0 + SG, :], ph[:, t0 : t0 + SG, :], msk_sb
                    )
                for v0 in range(0, V, SG):
                    ps = psp.tile([V, SG, T], F32, tag="sc", bufs=3)
                    for g in range(SG):
                        nc.tensor.matmul(
                            ps[:, g, :], k3[:, v0 + g, :], q3[:, v0 + g, :],
                            start=True, stop=True,
                        )
                    nc.scalar.activation(
                        out=pw[:, v0 : v0 + SG, :],
                        in_=ps.rearrange("p a b -> p (a b)"),
                        func=EXP,
                    )

                # ---- phase C: denominators and R ----
                pdh = psp.tile([V, T], F32, tag="sc", bufs=3)
                for t in range(T):
                    nc.tensor.matmul(
                        pdh[:, t : t + 1], ph[:, t, :], ones_sb,
                        start=True, stop=True,
                    )
                sh = den.tile([V, T], F32, tag="sh")
                nc.vector.tensor_copy(sh, pdh)
                pdw = psp.tile([T, V], F32, tag="sc", bufs=3)
                for v in range(V):
                    nc.tensor.matmul(
                        pdw[:, v : v + 1], pw[:, v, :], ones_sb,
                        start=True, stop=True,
                    )
                swt = den.tile([T, V], F32, tag="sw")
                nc.vector.tensor_copy(swt, pdw)
                ptr = psp.tile([V, T], F32, tag="sc", bufs=3)
                nc.tensor.transpose(ptr, swt, i96_sb)
                dv = den.tile([V, T], F32, tag="dv")
                nc.vector.tensor_add(dv, sh, ptr)
                rre = den.tile([V, T], F32, tag="rre")
                nc.vector.reciprocal(rre, dv)
                rvt = den.tile([V, T], F32, tag="rvt")
                nc.vector.tensor_scalar_mul(rvt, rre, gam_sb)
                ptr2 = psp.tile([T, V], F32, tag="sc", bufs=3)
                nc.tensor.transpose(ptr2, rvt, i96_sb)
                rtv = den.tile([T, V], F32, tag="rtv")
                nc.vector.tensor_copy(rtv, ptr2)

                # ---- phase D: outH (tiled by t, grouped by OG) ----
                vt_t = vtd[b].rearrange("(u t) c -> u t c", t=T)
                oh_t = oh_d[b].rearrange("(v t) c -> v t c", t=T)
                for t0 in range(0, T, OG):
                    vl = stg.tile([V, OG, C], BF16, tag="vtl", bufs=6)
                    nc.sync.dma_start(out=vl, in_=vt_t[:, t0 : t0 + OG, :])
                    os_ = stg.tile([V, OG, C], F16, tag="os")
                    for g in range(OG):
                        t = t0 + g
                        po = psp.tile([V, C], F32, tag="ob", bufs=2)
                        nc.tensor.matmul(
                            po, ph[:, t, :], vl[:, g, :], start=True, stop=True
                        )
                        if g % 2 == 0:
                            nc.vector.tensor_scalar_mul(
                                os_[:, g, :], po, rvt[:, t : t + 1]
                            )
                        else:
                            nc.scalar.activation(
                                out=os_[:, g, :], in_=po, func=COPY,
                                scale=rvt[:, t : t + 1],
                            )
                    nc.gpsimd.dma_start(out=oh_t[:, t0 : t0 + OG, :], in_=os_)

                # ---- phase E: outW (tiled by v, grouped by OG) ----
                ow_v = ow_d[b].rearrange("(v s) c -> s v c", s=T)
                for v0 in range(0, V, OG):
                    vl = stg.tile([T, OG, C], BF16, tag="vtl", bufs=6)
                    nc.sync.dma_start(
                        out=vl,
                        in_=vtd[b, v0 * T : (v0 + OG) * T, :].rearrange(
                            "(vb s) c -> s vb c", s=T
                        ),
                    )
                    os_ = stg.tile([T, OG, C], F16, tag="os")
                    for g in range(OG):
                        v = v0 + g
                        po = psp.tile([T, C], F32, tag="ob", bufs=2)
                        nc.tensor.matmul(
                            po, pw[:, v, :], vl[:, g, :], start=True, stop=True
                        )
                        if g % 2 == 0:
                            nc.vector.tensor_scalar_mul(
                                os_[:, g, :], po, rtv[:, v : v + 1]
                            )
                        else:
                            nc.scalar.activation(
                                out=os_[:, g, :], in_=po, func=COPY,
                                scale=rtv[:, v : v + 1],
                            )
                    nc.gpsimd.dma_start(
                        out=ow_v[:, v0 : v0 + OG, :], in_=os_
                    )

    nc.compile()
    return nc


def _prep_inputs(x, Wq, bq, Wk, bk, Wv, bv, gamma):
    x16 = np.ascontiguousarray(x.reshape(B, C, S)).astype(np.float16)
    wqk = np.concatenate([Wq, Wk], axis=0).T.reshape(NCI, 128, 128)
    wqk = np.ascontiguousarray(wqk).astype(np.float16)
    wv4 = np.ascontiguousarray(Wv.T.reshape(NCI, 128, C)).astype(np.float16)
    bqk = np.concatenate([bq, bk]).reshape(128, 1).astype(np.float32)
    bv2 = bv.reshape(1, C).astype(np.float32)
    gam = gamma.reshape(1, 1).astype(np.float32)
    mask = np.tile((1.0 - np.eye(V)), (1, SG)).astype(ml_dtypes.bfloat16)
    i96 = np.eye(V, dtype=np.float32)
    shared = {
        "wqk": wqk, "wv4": wv4, "bqk": bqk, "bv2": bv2, "gam": gam,
        "msk": mask, "i96": i96,
    }
    in_maps = [
        {"xh": x16[i * BPC : (i + 1) * BPC], **shared} for i in range(NCORES)
    ]
    return in_maps


def _run(inputs, trace=False, trace_kwargs=None):
    has_bv = bool(np.any(inputs["bv"]))
    has_bqk = bool(np.any(inputs["bq"])) or bool(np.any(inputs["bk"]))
    key = ("nc", has_bv, has_bqk)
    if key not in _CACHE:
        _CACHE[key] = _build(has_bv, has_bqk)
    nc = _CACHE[key]
    in_maps = _prep_inputs(**inputs)
    res = run_bass_kernel_spmd(
        nc, in_maps, list(range(NCORES)), trace=trace,
        **(trace_kwargs or {}),
    )
    oh = np.concatenate([res.results[i]["oh"] for i in range(NCORES)], axis=0)
    ow = np.concatenate([res.results[i]["ow"] for i in range(NCORES)], axis=0)
    part = oh.astype(np.float32) + ow.astype(np.float32)     # [B, S, C]
    out = inputs["x"].reshape(B, C, S) + part.transpose(0, 2, 1)
    return out.reshape(B, C, V, T).astype(np.float32), res


def kernel(**inputs):
    out, _ = _run(inputs)
    return out



# revision 23
# speedup vs baseline: 1.6459x; 1.6459x over previous
"""CrissCrossAttention TRN2 kernel (v6).

Shapes (hardcoded): x [16, 512, 96, 96] fp32, Wq/Wk [64, 512], Wv [512, 512],
biases, gamma [1]. 8 NeuronCores, data-parallel over batch (2 batches/core).

Math per batch (reference):
  q = Wq@x, k = Wk@x, val = Wv@x (1x1 convs over channel dim)
  eH[v,t,u] = sum_c q[c,v,t] k[c,u,t]  (diag u==v masked out)
  eW[v,t,s] = sum_c q[c,v,t] k[c,v,s]
  att = softmax(concat(eH, eW))  ->  out = gamma*(outH+outW) + x

Device strategy (per core, per batch) — compute layout as the well-tested
baseline (scores transposed so they are lhsT-ready, denominators via
ones-matmuls, normalization fused into PSUM evictions as per-partition
scales), plus:
  - STAGE PIPELINE: emission interleaves three tracks by estimated time so
    PE (projection matmuls), DMA (output streams) and Act/DVE (evictions,
    exp) stay concurrently busy:
      stage k =  B/C(k)  x  D/E(k-1)  x  A(k+1)
    qk and ph/pw are double-buffered to let adjacent batches overlap.
  - Loads on SP (HWDGE); ALL stores on GPSIMD (SWDGE) so dependent stores
    never head-of-line-block the load queue, and the Pool engine does
    nothing else.
  - Inputs merged into 3 tensors, both output streams interleaved in ONE
    [S, 2, C] tensor, to cut per-call dispatch overhead.  Host does
    (OH + OW).T + x.
"""

import numpy as np

import concourse.bacc as bacc
import concourse.tile as tile
from concourse import mybir
from concourse.bass_utils import run_bass_kernel_spmd

F32 = mybir.dt.float32
F16 = mybir.dt.float16
BF16 = mybir.dt.bfloat16
EXP = mybir.ActivationFunctionType.Exp
COPY = mybir.ActivationFunctionType.Copy
IDEN = mybir.ActivationFunctionType.Identity

B, C, V, T = 16, 512, 96, 96
C8 = 64
S = V * T            # 9216 spatial positions
NCORES = 8
BPC = B // NCORES    # batches per core
NCI = C // 128       # cin chunks
NJ = S // 512        # 512-wide column blocks
SG = 4               # score tiles grouped per PSUM bank
OG = 8               # out/load tiles grouped per DMA
DG = 12              # denominator column-chunk

_CACHE = {}


def _build(has_bv: bool, has_bqk: bool, repeat: int = 1):
    nc = bacc.Bacc("TRN2", target_bir_lowering=False, debug=False)

    xh = nc.dram_tensor("xh", [BPC, C, S], F16, kind="ExternalInput").ap()
    # wcat[:, :, 0:128] = [WqT|WkT] chunks, [:, :, 128:640] = WvT chunks
    wcat = nc.dram_tensor("wcat", [NCI, 128, 640], F16, kind="ExternalInput").ap()
    # mg[0:96, 0:384] = diag mask tiled x4, [0,384] = gamma, [:,385] = bqk,
    # [0, 392:904] = bv, [0:96, 904:1000] = eye(96)
    mg = nc.dram_tensor("mg", [128, 1000], F32, kind="ExternalInput").ap()
    o2_d = nc.dram_tensor("o2", [BPC, S, 2, C], F16, kind="ExternalOutput").ap()
    vtd = nc.dram_tensor("vtd", [BPC, S, C], BF16).ap()

    with tile.TileContext(nc) as tc:
        with (
            tc.tile_pool(name="const", bufs=1) as cst,
            tc.tile_pool(name="xts", bufs=4) as xts,
            tc.tile_pool(name="qkp", bufs=1) as qkp,
            tc.tile_pool(name="php", bufs=2) as php,
            tc.tile_pool(name="den", bufs=2) as den,
            tc.tile_pool(name="stg", bufs=2) as stg,
            tc.tile_pool(name="ps", bufs=1, space="PSUM") as psp,
        ):
            wqk_sb = cst.tile([128, NCI, 128], F16)
            nc.sync.dma_start(
                out=wqk_sb, in_=wcat.rearrange("a p b -> p a b")[:, :, 0:128]
            )
            wv_sb = cst.tile([128, NCI, C], F16)
            nc.sync.dma_start(
                out=wv_sb, in_=wcat.rearrange("a p b -> p a b")[:, :, 128:640]
            )
            mskf = cst.tile([V, SG * V], F32)
            nc.sync.dma_start(out=mskf, in_=mg[0:V, 0 : SG * V])
            msk_sb = cst.tile([V, SG, V], BF16)
            nc.vector.tensor_copy(msk_sb.rearrange("p a b -> p (a b)"), mskf)
            gam_sb = cst.tile([V, 1], F32)
            nc.sync.dma_start(out=gam_sb, in_=mg[0:1, 384:385].to_broadcast([V, 1]))
            i96_sb = cst.tile([V, V], F32)
            nc.sync.dma_start(out=i96_sb, in_=mg[0:V, 904:1000])
            ones_sb = cst.tile([V, 1], BF16)
            nc.vector.memset(ones_sb, 1.0)
            if has_bqk:
                bqk_sb = cst.tile([128, 1], F32)
                nc.sync.dma_start(out=bqk_sb, in_=mg[:, 385:386])
            if has_bv:
                bv_sb = cst.tile([128, C], F32)
                nc.sync.dma_start(
                    out=bv_sb, in_=mg[0:1, 392:904].to_broadcast([128, C])
                )

            rot = [0]

            def evict(out, in_, scale=None):
                # alternate PSUM evictions between Act and DVE; per-partition
                # scale (normalization) fuses for free on either engine
                if rot[0] % 2 == 0:
                    if scale is None:
                        nc.scalar.activation(out=out, in_=in_, func=COPY)
                    else:
                        nc.scalar.activation(
                            out=out, in_=in_, func=COPY, scale=scale
                        )
                else:
                    if scale is None:
                        nc.vector.tensor_copy(out, in_)
                    else:
                        nc.vector.tensor_scalar_mul(out, in_, scale)
                rot[0] += 1

            def emit_A_block(b, j, qk):
                """One 512-col block of phase A: x load, q|k proj, valT."""
                x_b = xh[b].rearrange("(ci p) s -> p ci s", p=128)
                vt_w = vtd[b].rearrange("(cb p) c -> p cb c", p=128)
                xt = xts.tile([128, NCI, 512], F16, tag="xt")
                nc.sync.dma_start(out=xt, in_=x_b[:, :, j * 512 : (j + 1) * 512])
                js = slice(j * 512, (j + 1) * 512)
                pq = psp.tile([128, 512], F32, tag="big", bufs=3)
                for ci in range(NCI):
                    nc.tensor.matmul(
                        pq, wqk_sb[:, ci, :], xt[:, ci, :],
                        start=(ci == 0), stop=(ci == NCI - 1),
                    )
                if has_bqk:
                    nc.scalar.activation(
                        out=qk[:, 0, js], in_=pq[0:C8, :], func=IDEN,
                        bias=bqk_sb[0:C8],
                    )
                    nc.scalar.activation(
                        out=qk[:, 1, js], in_=pq[C8:128, :], func=IDEN,
                        bias=bqk_sb[C8:128],
                    )
                else:
                    evict(qk[:, 0, js], pq[0:C8, :])
                    evict(qk[:, 1, js], pq[C8:128, :])
                vs = stg.tile([128, NCI, C], BF16, tag="vts", bufs=3)
                for jj in range(NCI):
                    pv = psp.tile([128, 512], F32, tag="big", bufs=3)
                    for ci in range(NCI):
                        nc.tensor.matmul(
                            pv,
                            xt[:, ci, jj * 128 : (jj + 1) * 128],
                            wv_sb[:, ci, :],
                            start=(ci == 0), stop=(ci == NCI - 1),
                        )
                    if has_bv:
                        nc.vector.tensor_add(vs[:, jj, :], pv, bv_sb)
                    else:
                        evict(vs[:, jj, :], pv)
                nc.gpsimd.dma_start(
                    out=vt_w[:, j * NCI : (j + 1) * NCI, :], in_=vs
                )

            def emit_BH_group(qk, ph, t0):
                """eH scores for 4 t's: matmul, exp, diag mask."""
                q3 = qk[:, 0, :].rearrange("p (v t) -> p v t", t=T)
                k3 = qk[:, 1, :].rearrange("p (v t) -> p v t", t=T)
                ps = psp.tile([V, SG, V], F32, tag="sc", bufs=3)
                for g in range(SG):
                    nc.tensor.matmul(
                        ps[:, g, :], k3[:, :, t0 + g], q3[:, :, t0 + g],
                        start=True, stop=True,
                    )
                nc.scalar.activation(
                    out=ph[:, t0 : t0 + SG, :],
                    in_=ps.rearrange("p a b -> p (a b)"),
                    func=EXP,
                )
                nc.vector.tensor_mul(
                    ph[:, t0 : t0 + SG, :], ph[:, t0 : t0 + SG, :], msk_sb
                )

            def emit_BW_group(qk, pw, v0):
                """eW scores for 4 v's: matmul, exp."""
                q3 = qk[:, 0, :].rearrange("p (v t) -> p v t", t=T)
                k3 = qk[:, 1, :].rearrange("p (v t) -> p v t", t=T)
                ps = psp.tile([V, SG, T], F32, tag="sc", bufs=3)
                for g in range(SG):
                    nc.tensor.matmul(
                        ps[:, g, :], k3[:, v0 + g, :], q3[:, v0 + g, :],
                        start=True, stop=True,
                    )
                nc.scalar.activation(
                    out=pw[:, v0 : v0 + SG, :],
                    in_=ps.rearrange("p a b -> p (a b)"),
                    func=EXP,
                )

            def emit_CH_chunk(ph, sh, hold, t0):
                """denH column sums for DG t's via ones-matmuls."""
                if t0 == 0:
                    psh_n = psp.tile([V, T], F32, tag="sc", bufs=3)
                    hold["h"] = psh_n
                psh = hold["h"]
                for t in range(t0, t0 + DG):
                    nc.tensor.matmul(
                        psh[:, t : t + 1], ph[:, t, :], ones_sb,
                        start=True, stop=True,
                    )
                if t0 + DG == T:
                    nc.vector.tensor_copy(sh, psh)

            def emit_CW_chunk(pw, swt, hold, v0):
                """denW column sums for DG v's via ones-matmuls."""
                if v0 == 0:
                    psw_n = psp.tile([T, V], F32, tag="sc", bufs=3)
                    hold["w"] = psw_n
                psw = hold["w"]
                for v in range(v0, v0 + DG):
                    nc.tensor.matmul(
                        psw[:, v : v + 1], pw[:, v, :], ones_sb,
                        start=True, stop=True,
                    )
                if v0 + DG == V:
                    nc.vector.tensor_copy(swt, psw)

            def emit_C_fin(sh, swt, rvt, rtv):
                """R = gamma/(denH+denW) in both [v,t] and [t,v] layouts."""
                ptr = psp.tile([V, T], F32, tag="sc", bufs=3)
                nc.tensor.transpose(ptr, swt, i96_sb)
                dv = den.tile([V, T], F32, tag="dv")
                nc.vector.tensor_add(dv, sh, ptr)
                rre = den.tile([V, T], F32, tag="rre")
                nc.vector.reciprocal(rre, dv)
                nc.vector.tensor_scalar_mul(rvt, rre, gam_sb)
                ptr2 = psp.tile([T, V], F32, tag="sc", bufs=3)
                nc.tensor.transpose(ptr2, rvt, i96_sb)
                nc.vector.tensor_copy(rtv, ptr2)

            def emit_D_block(b, ph, rvt, t0):
                """outH for OG t's: t-major valT load, matmuls, store."""
                vt_t = vtd[b].rearrange("(u t) c -> u t c", t=T)
                oh_t = o2_d[b, :, 0, :].rearrange("(v t) c -> v t c", t=T)
                vl = stg.tile([V, OG, C], BF16, tag="vtl", bufs=4)
                nc.sync.dma_start(out=vl, in_=vt_t[:, t0 : t0 + OG, :])
                os_ = stg.tile([V, OG, C], F16, tag="os", bufs=3)
                for g in range(OG):
                    t = t0 + g
                    po = psp.tile([V, C], F32, tag="ob", bufs=2)
                    nc.tensor.matmul(
                        po, ph[:, t, :], vl[:, g, :], start=True, stop=True
                    )
                    evict(os_[:, g, :], po, scale=rvt[:, t : t + 1])
                nc.gpsimd.dma_start(out=oh_t[:, t0 : t0 + OG, :], in_=os_)

            def emit_E_block(b, pw, rtv, v0):
                """outW for OG v's: v-major valT load, matmuls, store."""
                ow_v = o2_d[b, :, 1, :].rearrange("(v s) c -> s v c", s=T)
                vl = stg.tile([T, OG, C], BF16, tag="vtl", bufs=4)
                nc.sync.dma_start(
                    out=vl,
                    in_=vtd[b, v0 * T : (v0 + OG) * T, :].rearrange(
                        "(vb s) c -> s vb c", s=T
                    ),
                )
                os_ = stg.tile([T, OG, C], F16, tag="os", bufs=3)
                for g in range(OG):
                    v = v0 + g
                    po = psp.tile([T, C], F32, tag="ob", bufs=2)
                    nc.tensor.matmul(
                        po, pw[:, v, :], vl[:, g, :], start=True, stop=True
                    )
                    evict(os_[:, g, :], po, scale=rtv[:, v : v + 1])
                nc.gpsimd.dma_start(out=ow_v[:, v0 : v0 + OG, :], in_=os_)

            # ---- batch loop scheduling ----
            # Engines execute their queues IN ORDER, so interleaved emission
            # can head-of-line-block ready work behind stalled work.  The
            # schedule below stays sequential within phases and interleaves
            # only where the inserted units' deps are already satisfied:
            # phase A of batch k+1 (and optionally B/C of k+1 after that)
            # threaded into the DMA-bound D/E blocks of batch k.
            border = [bb for _ in range(repeat) for bb in range(BPC)]
            qk_t, ph_t, pw_t, r_t = {}, {}, {}, {}

            def units_A(bi):
                qkn = qkp.tile([C8, 2, S], F16, tag="qk")
                qk_t[bi] = qkn
                b = border[bi]
                return [
                    lambda j=j: emit_A_block(b, j, qkn) for j in range(NJ)
                ]

            def units_BC(bi):
                phn = php.tile([V, T, V], BF16, tag="ph")   # [u, t, v]
                pwn = php.tile([V, V, T], BF16, tag="pw")   # [s, v, t]
                sh = den.tile([V, T], F32, tag="sh")
                swt = den.tile([T, V], F32, tag="sw")
                rvt = den.tile([V, T], F32, tag="rvt")
                rtv = den.tile([T, V], F32, tag="rtv")
                ph_t[bi], pw_t[bi] = phn, pwn
                r_t[bi] = (rvt, rtv)
                qkn = qk_t[bi]
                us = [
                    lambda t0=t0: emit_BH_group(qkn, phn, t0)
                    for t0 in range(0, T, SG)
                ]
                us += [
                    lambda v0=v0: emit_BW_group(qkn, pwn, v0)
                    for v0 in range(0, V, SG)
                ]
                hold = {}
                us += [
                    lambda t0=t0: emit_CH_chunk(phn, sh, hold, t0)
                    for t0 in range(0, T, DG)
                ]
                us += [
                    lambda v0=v0: emit_CW_chunk(pwn, swt, hold, v0)
                    for v0 in range(0, V, DG)
                ]
                us += [lambda: emit_C_fin(sh, swt, rvt, rtv)]
                return us

            def units_DE(bi):
                b = border[bi]
                phn, pwn = ph_t[bi], pw_t[bi]
                rvt, rtv = r_t[bi]
                us = [
                    lambda t0=t0: emit_D_block(b, phn, rvt, t0)
                    for t0 in range(0, T, OG)
                ]
                us += [
                    lambda v0=v0: emit_E_block(b, pwn, rtv, v0)
                    for v0 in range(0, V, OG)
                ]
                return us

            import os as _os
                # SCHED 1: fully sequential (v1-style)
                # SCHED 2: A(k+1) interleaved into D/E(k), 1 A-block per 2
                # SCHED 3: like 2, plus B/C(k+1) units after A(k+1) runs dry
            SCHED = int(_os.environ.get("KSCHED", "3"))

            nb = len(border)
            if SCHED == 1:
                for k in range(nb):
                    for fn in units_A(k):
                        fn()
                    for fn in units_BC(k):
                        fn()
                    for fn in units_DE(k):
                        fn()
            else:
                pend = []       # leftover units carried into this batch
                for k in range(nb):
                    if k == 0:
                        for fn in units_A(0):
                            fn()
                        pend = units_BC(0)
                    for fn in pend:
                        fn()
                    pend = []
                    ins = units_A(k + 1) if k + 1 < nb else []
                    de = units_DE(k)
                    nde = len(de)
                    for i, fn in enumerate(de):
                        fn()
                        if i < len(ins):
                            # one A(k+1) block per DE block up front
                            ins[i]()
                        elif SCHED == 3 and k + 1 < nb:
                            if not pend and i == len(ins):
                                pend = units_BC(k + 1)
                            # spread remaining B/C units over leftover blocks
                            if pend:
                                left = nde - i
                                take = max(1, (len(pend) + left - 1) // left)
                                for _ in range(min(take, len(pend))):
                                    pend.pop(0)()
                    if SCHED == 2 and k + 1 < nb:
                        pend = units_BC(k + 1)

    nc.compile()
    return nc


def _prep_inputs(x, Wq, bq, Wk, bk, Wv, bv, gamma):
    x16 = np.ascontiguousarray(x.reshape(B, C, S)).astype(np.float16)
    wqk = np.concatenate([Wq, Wk], axis=0).T.reshape(NCI, 128, 128)
    wv4 = Wv.T.reshape(NCI, 128, C)
    wcat = np.ascontiguousarray(
        np.concatenate([wqk, wv4], axis=2).astype(np.float16)
    )
    mg = np.zeros((128, 1000), np.float32)
    mg[0:V, 0 : SG * V] = np.tile(1.0 - np.eye(V), (1, SG))
    mg[0, 384] = gamma.reshape(-1)[0]
    mg[:, 385] = np.concatenate([bq, bk])
    mg[0, 392:904] = bv
    mg[0:V, 904:1000] = np.eye(V)
    shared = {"wcat": wcat, "mg": mg}
    in_maps = [
        {"xh": x16[i * BPC : (i + 1) * BPC], **shared} for i in range(NCORES)
    ]
    return in_maps


def _run(inputs, trace=False, trace_kwargs=None):
    has_bv = bool(np.any(inputs["bv"]))
    has_bqk = bool(np.any(inputs["bq"])) or bool(np.any(inputs["bk"]))
    key = ("nc", has_bv, has_bqk)
    if key not in _CACHE:
        _CACHE[key] = _build(has_bv, has_bqk)
    nc = _CACHE[key]
    in_maps = _prep_inputs(**inputs)
    res = run_bass_kernel_spmd(
        nc, in_maps, list(range(NCORES)), trace=trace,
        **(trace_kwargs or {}),
    )
    o2 = np.concatenate([res.results[i]["o2"] for i in range(NCORES)], axis=0)
    part = o2[:, :, 0, :].astype(np.float32) + o2[:, :, 1, :].astype(np.float32)
    out = inputs["x"].reshape(B, C, S) + part.transpose(0, 2, 1)
    return out.reshape(B, C, V, T).astype(np.float32), res


def kernel(**inputs):
    out, _ = _run(inputs)
    return out
